# revision 34
# baseline (speedup 1.0000x reference)
"""Masked-reconstruction (stem->conv->GRU->head->masked MSE) Bass kernel.

v2: chunk-parallel GRU scan with warmup.

Per core (B_C=8 batch rows):
  Phase A encoder: row-pairs stacked on 128 partitions, bf16 matmuls.
    xmT (pre-masked x, bf16, host) -> stem -> gelu -> conv(k3) -> gelu
    -> convh DRAM [B_C, 65, T] bf16 (row 64 = ones for the gx bias trick).
  Phase B scan: each row split into C=48 chunks; chunk j covers
    [j*K, j*K+L) with K=85, W=16 warmup, L=K+W=101. All chunks start
    from h=0; outputs valid for local steps >=W (j=0: all steps).
    Lanes grouped G=3 x NB=128, groups software-pipelined per step.
    gx prefilled into PSUM via PE (K=65 with ones-row bias), recurrent
    matmuls accumulate on top; n-gate r*(ghn+bhhn) via one DVE STT, gxn
    accumulate via PE identity matmul, h-update split DVE/Pool.
    hout [128, G, L, NB] bf16 stays in SBUF.
  Phase C head: 3-layer MLP read from hout SBUF; loss via host-packed
    mxb = m*(x - h3_b) and m tiles; sf = sum_t (m*p3 - mxb)^2 per
    feature. Host finalizes: loss = sum_f sf_f/scale_f^2 / max(sum m,1).
"""
import os
from contextlib import ExitStack

import numpy as np
import ml_dtypes

import bass_rust
import concourse.bass as bass
import concourse.mybir as mybir
import concourse.tile as tile
from concourse import bacc
from concourse.bass import ts


def _vecpairs(dims):
    return bass_rust.VecI64Pair([list(d) for d in dims])

F32 = mybir.dt.float32
BF16 = mybir.dt.bfloat16
F8 = mybir.dt.float8e4
AF = mybir.ActivationFunctionType
ALU = mybir.AluOpType
BF = ml_dtypes.bfloat16
F8NP = ml_dtypes.float8_e4m3

B, F, DH, DG = 64, 64, 64, 128
NCORE = 8
B_C = B // NCORE   # 8 batch rows per core
T = 4096
CHK = 48           # chunks per row
W = 16             # warmup steps
K = (T - W) // CHK  # 85 output cols per chunk (j=0 gets K+W)
L = K + W          # 101 lockstep scan steps
G = 3              # lane groups (software pipeline)
NB = 128           # lanes per group = 16 chunks x 8 rows
JPG = NB // B_C    # 16 chunks per group
TILE_T = 512
NT = T // TILE_T
IMM_ON_PE = True
# head tiling: blocks of HL lanes x K cols
HL = 6
NHB = (NB + HL - 1) // HL      # 22 blocks per group (last has 2 lanes)
WMAX = HL * K                  # 510
GBC = 11 * WMAX + (NB - 21 * HL) * K  # block-cols per group: 10 pairs + 2 singles
EXTRA_W = B_C * W
TOTBC = G * GBC + EXTRA_W
NTILES = G * 12 + 1            # 10 pairs + 2 singles per group + extra


def _tile_cols():
    """Per-tile (b, t) column lists in device iteration order.
    Main tiles: per group, nb-blocks of HL lanes x t in [W, L); lane nb ->
    b = nb // JPG, j = g*JPG + nb % JPG, position j*K + t.
    Extra tile: j=0 lanes (g=0, nb % JPG == 0), t in [0, W)."""
    tiles = []
    for g in range(G):
        gt = []
        for nb0 in range(0, NB, HL):
            cols = []
            for nb in range(nb0, min(nb0 + HL, NB)):
                b = nb // JPG
                j = g * JPG + nb % JPG
                cols += [(b, j * K + t) for t in range(W, L)]
            gt.append(np.asarray(cols))
        tiles.append(gt)
    extra = np.asarray([(b, t) for b in range(B_C) for t in range(W)])
    return tiles, extra


def _pack_mmx(mf, mxb):
    """Build mmxP [2, 128, TOT]: per group, 10 stacked pairs + 2 singles,
    then the j0-extra single. Singles occupy rows 0:64 (rows 64:128 zero);
    short blocks zero-padded (m=0 kills garbage columns)."""
    tiles, extra = _tile_cols()
    blocks_m, blocks_x = [], []

    def grab(arr, cols):
        return arr[cols[:, 0], cols[:, 1]].T  # [64, n]

    def block(cols_a, cols_b, width):
        bm = np.zeros((128, width), np.float32)
        bx = np.zeros((128, width), np.float32)
        bm[0:64, :len(cols_a)] = grab(mf, cols_a)
        bx[0:64, :len(cols_a)] = grab(mxb, cols_a)
        if cols_b is not None:
            bm[64:128, :len(cols_b)] = grab(mf, cols_b)
            bx[64:128, :len(cols_b)] = grab(mxb, cols_b)
        blocks_m.append(bm)
        blocks_x.append(bx)

    wmax = HL * K
    for g in range(G):
        gt = tiles[g]
        for i in range(10):
            block(gt[2 * i], gt[2 * i + 1], wmax)
        block(gt[20], None, wmax)
        block(gt[21], None, len(gt[21]))
    block(extra, None, len(extra))
    m = np.concatenate(blocks_m, axis=1)
    x = np.concatenate(blocks_x, axis=1)
    return np.ascontiguousarray(np.stack([m, x])).astype(BF)


def prep_inputs(inputs):
    """Host-side layout prep. Returns (per-core input dicts, host_ctx)."""
    x = np.asarray(inputs["x"], np.float32)
    fm = np.asarray(inputs["feature_mask"])

    w = {}
    sw = np.asarray(inputs["stem_w"], np.float32)
    w["stemW"] = np.ascontiguousarray(np.concatenate([sw, sw], axis=0)).astype(BF)  # (2F, DH) lhsT x2
    stem_b = np.asarray(inputs["stem_b"], np.float32)
    w["stemB2"] = np.ascontiguousarray(np.concatenate([stem_b, stem_b]).reshape(128, 1))
    cw = np.asarray(inputs["conv_w"], np.float32)  # (out, in, 3)
    cwt = cw.transpose(2, 1, 0)
    w["convW"] = np.ascontiguousarray(np.concatenate([cwt, cwt], axis=1)).astype(BF)  # (3, 2*in, out)
    conv_b = np.asarray(inputs["conv_b"], np.float32)
    w["convB2"] = np.ascontiguousarray(np.concatenate([conv_b, conv_b]).reshape(128, 1))
    wih = np.asarray(inputs["gru_w_ih"], np.float32)
    whh = np.asarray(inputs["gru_w_hh"], np.float32)
    bih = np.asarray(inputs["gru_b_ih"], np.float32)
    bhh = np.asarray(inputs["gru_b_hh"], np.float32)
    wihT = np.zeros((3, DH + 1, DG), np.float32)
    for g in range(3):
        wihT[g, :DH] = wih[g * DG:(g + 1) * DG].T
        bias = bih[g * DG:(g + 1) * DG].copy()
        if g < 2:
            bias += bhh[g * DG:(g + 1) * DG]
        wihT[g, DH] = bias
    w["wihT"] = wihT.astype(F8NP)
    w["whhT"] = np.ascontiguousarray(
        np.stack([whh[g * DG:(g + 1) * DG].T for g in range(3)])).astype(BF)
    w["bhhn"] = bhh[2 * DG:].reshape(DG, 1).astype(np.float32)
    w["ident"] = np.eye(128, dtype=np.float32).astype(BF)
    w["h1w"] = np.asarray(inputs["h1_w"], np.float32).astype(BF)
    w["h1b"] = np.asarray(inputs["h1_b"], np.float32).reshape(128, 1)
    w["h2w"] = np.asarray(inputs["h2_w"], np.float32).astype(BF)
    w["h2b"] = np.asarray(inputs["h2_b"], np.float32).reshape(128, 1)
    w["h3w"] = np.asarray(inputs["h3_w"], np.float32).astype(BF)  # (128, 64)
    h3b = np.asarray(inputs["h3_b"], np.float32)

    per_core = []
    for c in range(NCORE):
        rows = slice(c * B_C, (c + 1) * B_C)
        xc = x[rows]                       # (8, T, F)
        fmc = fm[rows]
        xm = np.where(fmc, 0.0, xc)
        d = dict(w)
        d["xmT"] = np.ascontiguousarray(xm.transpose(0, 2, 1)).astype(BF)
        mf = fmc.astype(np.float32)
        mxb = mf * (xc - h3b)              # (8, T, F)
        d["mmxP"] = _pack_mmx(mf, mxb)
        per_core.append(d)

    scale = np.std(x.astype(np.float64), axis=(0, 1), ddof=1) + 1e-8
    host_ctx = dict(inv_scale2=1.0 / (scale * scale),
                    msum=float(fm.sum()))
    return per_core, host_ctx


def host_finalize(core_outs, host_ctx):
    tot = np.sum([np.asarray(o, np.float64).sum(axis=1) for o in core_outs], axis=0)
    sf = tot[0:64] + tot[64:128]
    num = float(np.sum(sf * host_ctx["inv_scale2"]))
    den = max(host_ctx["msum"], 1.0)
    return np.float32(num / den)


def build_program():
    nc = bacc.Bacc("TRN2", target_bir_lowering=False, debug=False,
                   num_devices=NCORE)

    # ---- DRAM tensors ----
    xmT = nc.dram_tensor("xmT", [B_C, F, T], BF16, kind="ExternalInput").ap()
    stemW = nc.dram_tensor("stemW", [2 * F, DH], BF16, kind="ExternalInput").ap()
    stemB2 = nc.dram_tensor("stemB2", [128, 1], F32, kind="ExternalInput").ap()
    convW = nc.dram_tensor("convW", [3, 2 * DH, DH], BF16, kind="ExternalInput").ap()
    convB2 = nc.dram_tensor("convB2", [128, 1], F32, kind="ExternalInput").ap()
    wihT = nc.dram_tensor("wihT", [3, DH + 1, DG], F8, kind="ExternalInput").ap()
    whhT = nc.dram_tensor("whhT", [3, DG, DG], BF16, kind="ExternalInput").ap()
    bhhn = nc.dram_tensor("bhhn", [DG, 1], F32, kind="ExternalInput").ap()
    ident = nc.dram_tensor("ident", [128, 128], BF16, kind="ExternalInput").ap()
    h1w = nc.dram_tensor("h1w", [DG, 128], BF16, kind="ExternalInput").ap()
    h1b = nc.dram_tensor("h1b", [128, 1], F32, kind="ExternalInput").ap()
    h2w = nc.dram_tensor("h2w", [128, 128], BF16, kind="ExternalInput").ap()
    h2b = nc.dram_tensor("h2b", [128, 1], F32, kind="ExternalInput").ap()
    h3w = nc.dram_tensor("h3w", [128, F], BF16, kind="ExternalInput").ap()
    mmxP = nc.dram_tensor("mmxP", [2, 128, TOTBC], BF16, kind="ExternalInput").ap()
    convh = nc.dram_tensor("convh", [B_C, DH + 1, T], F8).ap()
    out = nc.dram_tensor("out", [128, NTILES], F32, kind="ExternalOutput").ap()

    with tile.TileContext(nc) as tc, ExitStack() as ctx:
        wpool = ctx.enter_context(tc.tile_pool(name="weights", bufs=1))
        spool = ctx.enter_context(tc.tile_pool(name="stats", bufs=1))

        def wtile(shape, src, tag, dt=BF16):
            t = wpool.tile(shape, dt, tag=tag)
            nc.sync.dma_start(t[:], src)
            return t

        stemW_s = wtile([2 * F, DH], stemW[:], "w_stemW")
        stemB2_s = wtile([128, 1], stemB2[:], "w_stemB2", F32)
        convW_s = [wtile([2 * DH, DH], convW[dt], f"w_convW{dt}") for dt in range(3)]
        convB2_s = wtile([128, 1], convB2[:], "w_convB2", F32)
        wih_s = [wtile([DH + 1, DG], wihT[g], f"w_wih{g}", F8) for g in range(3)]
        whh_s = [wtile([DG, DG], whhT[g], f"w_whh{g}") for g in range(3)]
        bhhn_s = wtile([DG, 1], bhhn[:], "w_bhhn", F32)
        ident_s = wtile([128, 128], ident[:], "w_ident")
        h1w_s = wtile([DG, 128], h1w[:], "w_h1w")
        h1b_s = wtile([128, 1], h1b[:], "w_h1b", F32)
        h2w_s = wtile([128, 128], h2w[:], "w_h2w")
        h2b_s = wtile([128, 1], h2b[:], "w_h2b", F32)
        h3w_s = wtile([128, F], h3w[:], "w_h3w")

        # persistent slabs
        hout = wpool.tile([128, G, L, NB], BF16, tag="hout")
        zlane = wpool.tile([128, NB], BF16, tag="zlane")
        nc.vector.memset(zlane[:], 0.0)
        ones_sb = wpool.tile([128, T // 128], F8, tag="ones_sb")
        nc.vector.memset(ones_sb[:], 1.0)

        st_sf = spool.tile([128, NTILES], F32)
        nc.vector.memset(st_sf[:], 0.0)

        # ================= Phase A: encoder =================
        with tc.tile_pool(name="enc_io", bufs=3) as io, \
             tc.tile_pool(name="enc_row", bufs=1) as rowp, \
             tc.tile_pool(name="enc_ps", bufs=2, space="PSUM") as eps:
            for b in range(B_C):
                # ones row for the gx bias trick
                oview = convh[b, DH, 0:1].copy()
                oview.ap = _vecpairs([[T // 128, 128], [1, T // 128]])
                nc.sync.dma_start(oview, ones_sb[:])
            CHB = 4 * TILE_T  # conv write burst (4 tiles)
            hrows, xslabs, chslabs = [], [], []
            for p in range(B_C // 2):
                hr = rowp.tile([128, T + 2], BF16, tag=f"hrow{p}", name=f"hrow{p}")
                nc.vector.memset(hr[:, 0:1], 0.0)
                nc.vector.memset(hr[:, T + 1:T + 2], 0.0)
                hrows.append(hr)
                xs = rowp.tile([128, T], BF16, tag=f"xslab{p}", name=f"xslab{p}")
                nc.sync.dma_start(xs[:], xmT[2 * p:2 * p + 2, :, :].rearrange("b c t -> (b c) t"))
                xslabs.append(xs)

            def conv_tile(p, it, chs):
                ps = eps.tile([128, TILE_T], F32, tag="conv_ps")
                for dt in range(3):
                    csl = slice(it * TILE_T + dt, it * TILE_T + dt + TILE_T)
                    nc.tensor.matmul(ps[0:64], convW_s[dt][0:64], hrows[p][0:64, csl],
                                     start=(dt == 0), stop=(dt == 2))
                    nc.tensor.matmul(ps[64:128], convW_s[dt][64:128], hrows[p][64:128, csl],
                                     start=(dt == 0), stop=(dt == 2))
                nc.scalar.activation(chs[:, (it % 4) * TILE_T:(it % 4 + 1) * TILE_T],
                                     ps[:], AF.Gelu, bias=convB2_s[:])
                if it % 4 == 3:
                    t0 = (it - 3) * TILE_T
                    nc.sync.dma_start(convh[2 * p, 0:DH, t0:t0 + CHB], chs[0:64])
                    nc.sync.dma_start(convh[2 * p + 1, 0:DH, t0:t0 + CHB], chs[64:128])

            chcur = [None] * (B_C // 2)
            for it in range(NT + 1):
                if it < NT:
                    tsl = ts(it, TILE_T)
                    for p in range(B_C // 2):
                        ps = eps.tile([128, TILE_T], F32, tag="stem_ps")
                        nc.tensor.matmul(ps[0:64], stemW_s[0:64], xslabs[p][0:64, tsl],
                                         start=True, stop=True)
                        nc.tensor.matmul(ps[64:128], stemW_s[64:128], xslabs[p][64:128, tsl],
                                         start=True, stop=True)
                        nc.scalar.activation(hrows[p][:, 1 + it * TILE_T:1 + (it + 1) * TILE_T],
                                             ps[:], AF.Gelu, bias=stemB2_s[:])
                if it >= 1:
                    for p in range(B_C // 2):
                        if (it - 1) % 4 == 0:
                            chcur[p] = rowp.tile([128, CHB], F8, tag=f"chslab{p}",
                                                 name=f"chs{p}_{it}", bufs=2)
                        conv_tile(p, it - 1, chcur[p])

        # ================= Phase B: chunk-parallel scan =================
        CW = L  # cin slab covers the whole scan
        with tc.tile_pool(name="scan_cin", bufs=1) as sin, \
             tc.tile_pool(name="scan_sm", bufs=3) as ssm, \
             tc.tile_pool(name="scan_ps", bufs=1, space="PSUM") as sps:
            # PSUM layout per group: [128, parity, slot, NB] f32 (2KB = 1 bank)
            P_rz = [sps.tile([128, 2, 2, NB], F32, tag=f"prz{g}", name=f"prz{g}") for g in range(G)]
            P_ngx = [sps.tile([128, 2, 2, NB], F32, tag=f"pngx{g}", name=f"pngx{g}") for g in range(G)]
            for g in range(G):
                nc.vector.memset(P_ngx[g][:, 0, 0, :], 0.0)  # P_n zero for t=0

            cins = [None] * G

            def load_cin(g):
                cin = wpool.tile([DH + 1, B_C, JPG, CW], F8, tag=f"cin{g}", name=f"cin{g}")
                for b in range(B_C):
                    src = convh[b, 0:DH + 1, 0:1].copy()
                    src.ap = _vecpairs([[T, DH + 1], [K, JPG], [1, L]])
                    src.offset = b * (DH + 1) * T + g * JPG * K
                    nc.sync.dma_start(cin[:, b, :, :], src)
                return cin

            def prefill(g, t):
                tp = t % 2
                if t == 0:
                    cins[g] = load_cin(g)
                rhs = cins[g][:, :, :, t]
                nc.tensor.matmul(P_rz[g][:, tp, 0, :], wih_s[0][:], rhs,
                                 start=True, stop=True, skip_group_check=True)
                nc.tensor.matmul(P_rz[g][:, tp, 1, :], wih_s[1][:], rhs,
                                 start=True, stop=True, skip_group_check=True)
                nc.tensor.matmul(P_ngx[g][:, tp, 1, :], wih_s[2][:], rhs,
                                 start=True, stop=True, skip_group_check=True)

            for g in range(G):
                prefill(g, 0)
            prev_n = [None] * G
            prev_zd = [None] * G
            for t in range(L):
                cp = t % 2
                if t + 1 < L:
                    for g in range(G):
                        prefill(g, t + 1)
                if t > 0:
                    for g in range(G):
                        # recurrent state fed as two operands: h = n + zd
                        nc.tensor.matmul(P_rz[g][:, cp, 0, :], whh_s[0][:], prev_n[g][:],
                                         start=False, stop=True, skip_group_check=True)
                        nc.tensor.matmul(P_rz[g][:, cp, 1, :], whh_s[1][:], prev_n[g][:],
                                         start=False, stop=True, skip_group_check=True)
                        nc.tensor.matmul(P_ngx[g][:, cp, 0, :], whh_s[2][:], prev_n[g][:],
                                         start=True, stop=False, skip_group_check=True)
                        nc.tensor.matmul(P_rz[g][:, cp, 0, :], whh_s[0][:], prev_zd[g][:],
                                         start=False, stop=True, skip_group_check=True)
                        nc.tensor.matmul(P_rz[g][:, cp, 1, :], whh_s[1][:], prev_zd[g][:],
                                         start=False, stop=True, skip_group_check=True)
                        nc.tensor.matmul(P_ngx[g][:, cp, 0, :], whh_s[2][:], prev_zd[g][:],
                                         start=False, stop=True, skip_group_check=True)
                for g in range(G):
                    h_prev = zlane[:] if t == 0 else hout[:, g, t - 1, :]
                    rz_sb = ssm.tile([128, 2, NB], BF16, tag=f"rz{g}")
                    nc.scalar.activation(rz_sb[:], P_rz[g][:, cp, :, :], AF.Sigmoid)
                    # tmp = (P_n + bhhn) * r
                    tmp = ssm.tile([128, NB], BF16, tag=f"tmp{g}")
                    nc.vector.scalar_tensor_tensor(tmp[:], P_ngx[g][:, cp, 0, :],
                                                   bhhn_s[:], rz_sb[:, 0, :],
                                                   ALU.add, ALU.mult)
                    if IMM_ON_PE:
                        nc.tensor.matmul(P_ngx[g][:, cp, 1, :], ident_s[:], tmp[:],
                                         start=False, stop=True, skip_group_check=True)
                    else:
                        nc.vector.tensor_add(P_ngx[g][:, cp, 1, :],
                                             P_ngx[g][:, cp, 1, :], tmp[:])
                    n_sb = ssm.tile([128, NB], BF16, tag=f"n{g}")
                    nc.scalar.activation(n_sb[:], P_ngx[g][:, cp, 1, :], AF.Tanh)
                    d_sb = ssm.tile([128, NB], BF16, tag=f"d{g}")
                    nc.vector.tensor_sub(d_sb[:], h_prev, n_sb[:])
                    zd_sb = ssm.tile([128, NB], BF16, tag=f"zd{g}")
                    nc.vector.tensor_mul(zd_sb[:], rz_sb[:, 1, :], d_sb[:])
                    nc.gpsimd.tensor_tensor(hout[:, g, t, :], n_sb[:], zd_sb[:], ALU.add)
                    prev_n[g], prev_zd[g] = n_sb, zd_sb

        # ================= Phase C: head + loss =================
        with tc.tile_pool(name="head_io", bufs=3) as hio, \
             tc.tile_pool(name="head_tmp", bufs=3) as htmp, \
             tc.tile_pool(name="head_ps", bufs=2, space="PSUM") as hps:
            def mlp(zview, ncols, p3dst):
                '''h1->gelu->h2->gelu->h3 into the given PSUM [64-row] dest.'''
                p1 = hps.tile([128, WMAX], F32, tag="p1")
                nc.tensor.matmul(p1[:, 0:ncols], h1w_s[:], zview, start=True, stop=True)
                r1 = htmp.tile([128, WMAX], BF16, tag="r1")
                nc.scalar.activation(r1[:, 0:ncols], p1[:, 0:ncols], AF.Gelu, bias=h1b_s[:])
                p2 = hps.tile([128, WMAX], F32, tag="p2")
                nc.tensor.matmul(p2[:, 0:ncols], h2w_s[:], r1[:, 0:ncols], start=True, stop=True)
                r2 = htmp.tile([128, WMAX], BF16, tag="r2")
                nc.scalar.activation(r2[:, 0:ncols], p2[:, 0:ncols], AF.Gelu, bias=h2b_s[:])
                nc.tensor.matmul(p3dst, h3w_s[:], r2[:, 0:ncols], start=True, stop=True)

            def loss_block(p3, width, mslab, moff, tidx):
                t1 = htmp.tile([128, WMAX], F32, tag="t1")
                nc.vector.tensor_mul(t1[:, 0:width], p3, mslab[:, 0, :, moff:moff + width].squeeze())
                dd = htmp.tile([128, WMAX], BF16, tag="dd")
                nc.vector.tensor_sub(dd[:, 0:width], t1[:, 0:width],
                                     mslab[:, 1, :, moff:moff + width].squeeze())
                sq = htmp.tile([128, WMAX], BF16, tag="junk")
                nc.scalar.activation(sq[:, 0:width], dd[:, 0:width], AF.Square,
                                     accum_out=st_sf[:, tidx:tidx + 1])

            def zview_of(g, nb0, nl):
                return hout[:, g, W:L, nb0:nb0 + nl].rearrange("p t n -> p n t")

            tidx = 0
            for g in range(G):
                gw = GBC + (EXTRA_W if g == G - 1 else 0)
                mslab = hio.tile([128, 2, 1, gw], BF16, tag="mslab", bufs=2)
                nc.sync.dma_start(mslab[:], mmxP[:, :, g * GBC:g * GBC + gw].rearrange(
                    "a p c -> p a () c"))
                moff = 0
                for i in range(10):  # stacked pairs
                    p3 = hps.tile([128, WMAX], F32, tag="p3")
                    mlp(zview_of(g, 2 * i * HL, HL), WMAX, p3[0:64, :])
                    mlp(zview_of(g, (2 * i + 1) * HL, HL), WMAX, p3[64:128, :])
                    loss_block(p3[:], WMAX, mslab, moff, tidx)
                    moff += WMAX
                    tidx += 1
                # singles: block 20 (full) and block 21 (short)
                for nb0, wsub in ((20 * HL, WMAX), (21 * HL, (NB - 21 * HL) * K)):
                    p3 = hps.tile([128, WMAX], F32, tag="p3")
                    nc.vector.memset(p3[64:128, 0:wsub], 0.0)
                    mlp(zview_of(g, nb0, (wsub // K)), wsub, p3[0:64, 0:wsub])
                    loss_block(p3[:, 0:wsub], wsub, mslab, moff, tidx)
                    moff += wsub
                    tidx += 1
            # j0-extra single
            p3 = hps.tile([128, WMAX], F32, tag="p3")
            nc.vector.memset(p3[64:128, 0:EXTRA_W], 0.0)
            zv = hout[:, 0, 0, 0:1].copy()
            zv.ap = _vecpairs([list(hout.ap[0]), [JPG, B_C], [NB, W]])
            zv.offset = hout.offset
            mlp(zv, EXTRA_W, p3[0:64, 0:EXTRA_W])
            loss_block(p3[:, 0:EXTRA_W], EXTRA_W, mslab, moff, tidx)
            tidx += 1
            nc.sync.dma_start(out[:], st_sf[:])

    nc.compile()
    return nc


_CACHE = {}


def kernel(**inputs):
    """Full-input entry point: shards over 8 NeuronCores, runs the Bass
    program, returns the scalar loss (np.float32)."""
    from concourse.bass_utils import run_bass_kernel_spmd

    if "nc" not in _CACHE:
        _CACHE["nc"] = build_program()
    nc = _CACHE["nc"]
    per_core, host_ctx = prep_inputs(inputs)
    res = run_bass_kernel_spmd(nc, per_core, list(range(NCORE))).results
    return host_finalize([r["out"] for r in res], host_ctx)


# revision 35
# speedup vs baseline: 1.2567x; 1.2567x over previous
"""Masked-reconstruction (stem->conv->GRU->head->masked MSE) Bass kernel.

v2: chunk-parallel GRU scan with warmup.

Per core (B_C=8 batch rows):
  Phase A encoder: row-pairs stacked on 128 partitions, bf16 matmuls.
    xmT (pre-masked x, bf16, host) -> stem -> gelu -> conv(k3) -> gelu
    -> convh DRAM [B_C, 65, T] bf16 (row 64 = ones for the gx bias trick).
  Phase B scan: each row split into C=48 chunks; chunk j covers
    [j*K, j*K+L) with K=85, W=16 warmup, L=K+W=101. All chunks start
    from h=0; outputs valid for local steps >=W (j=0: all steps).
    Lanes grouped G=3 x NB=128, groups software-pipelined per step.
    gx prefilled into PSUM via PE (K=65 with ones-row bias), recurrent
    matmuls accumulate on top; n-gate r*(ghn+bhhn) via one DVE STT, gxn
    accumulate via PE identity matmul, h-update split DVE/Pool.
    hout [128, G, L, NB] bf16 stays in SBUF.
  Phase C head: 3-layer MLP read from hout SBUF; loss via host-packed
    mxb = m*(x - h3_b) and m tiles; sf = sum_t (m*p3 - mxb)^2 per
    feature. Host finalizes: loss = sum_f sf_f/scale_f^2 / max(sum m,1).
"""
import os
from contextlib import ExitStack

import numpy as np
import ml_dtypes

import bass_rust
import concourse.bass as bass
import concourse.mybir as mybir
import concourse.tile as tile
from concourse import bacc
from concourse.bass import ts


def _vecpairs(dims):
    return bass_rust.VecI64Pair([list(d) for d in dims])

F32 = mybir.dt.float32
BF16 = mybir.dt.bfloat16
F8 = mybir.dt.float8e4
AF = mybir.ActivationFunctionType
ALU = mybir.AluOpType
BF = ml_dtypes.bfloat16
F8NP = ml_dtypes.float8_e4m3

B, F, DH, DG = 64, 64, 64, 128
NCORE = 8
B_C = B // NCORE   # 8 batch rows per core
T = 4096
CHK = 48           # chunks per row
W = 16             # warmup steps
K = (T - W) // CHK  # 85 output cols per chunk (j=0 gets K+W)
L = K + W          # 101 lockstep scan steps
G = 3              # lane groups (software pipeline)
NB = 128           # lanes per group = 16 chunks x 8 rows
JPG = NB // B_C    # 16 chunks per group
TILE_T = 512
NT = T // TILE_T
IMM_ON_PE = True
# head tiling: blocks of HL lanes x K cols
HL = 6
NHB = (NB + HL - 1) // HL      # 22 blocks per group (last has 2 lanes)
WMAX = HL * K                  # 510
GBC = 11 * WMAX + (NB - 21 * HL) * K  # block-cols per group: 10 pairs + 2 singles
EXTRA_W = B_C * W
TOTBC = G * GBC + EXTRA_W
NTILES = G * 12 + 1            # 10 pairs + 2 singles per group + extra


def _tile_cols():
    """Per-tile (b, t) column lists in device iteration order.
    Main tiles: per group, nb-blocks of HL lanes x t in [W, L); lane nb ->
    b = nb // JPG, j = g*JPG + nb % JPG, position j*K + t.
    Extra tile: j=0 lanes (g=0, nb % JPG == 0), t in [0, W)."""
    tiles = []
    for g in range(G):
        gt = []
        for nb0 in range(0, NB, HL):
            cols = []
            for nb in range(nb0, min(nb0 + HL, NB)):
                b = nb // JPG
                j = g * JPG + nb % JPG
                cols += [(b, j * K + t) for t in range(W, L)]
            gt.append(np.asarray(cols))
        tiles.append(gt)
    extra = np.asarray([(b, t) for b in range(B_C) for t in range(W)])
    return tiles, extra


def _pack_mmx(mf, mxb):
    """Build mmxP [2, 128, TOT]: per group, 10 stacked pairs + 2 singles,
    then the j0-extra single. Singles occupy rows 0:64 (rows 64:128 zero);
    short blocks zero-padded (m=0 kills garbage columns)."""
    tiles, extra = _tile_cols()
    blocks_m, blocks_x = [], []

    def grab(arr, cols):
        return arr[cols[:, 0], cols[:, 1]].T  # [64, n]

    def block(cols_a, cols_b, width):
        bm = np.zeros((128, width), np.float32)
        bx = np.zeros((128, width), np.float32)
        bm[0:64, :len(cols_a)] = grab(mf, cols_a)
        bx[0:64, :len(cols_a)] = grab(mxb, cols_a)
        if cols_b is not None:
            bm[64:128, :len(cols_b)] = grab(mf, cols_b)
            bx[64:128, :len(cols_b)] = grab(mxb, cols_b)
        blocks_m.append(bm)
        blocks_x.append(bx)

    wmax = HL * K
    for g in range(G):
        gt = tiles[g]
        for i in range(10):
            block(gt[2 * i], gt[2 * i + 1], wmax)
        block(gt[20], None, wmax)
        block(gt[21], None, len(gt[21]))
    block(extra, None, len(extra))
    m = np.concatenate(blocks_m, axis=1)
    x = np.concatenate(blocks_x, axis=1)
    return np.ascontiguousarray(np.stack([m, x])).astype(BF)


def prep_inputs(inputs):
    """Host-side layout prep. Returns (per-core input dicts, host_ctx)."""
    x = np.asarray(inputs["x"], np.float32)
    fm = np.asarray(inputs["feature_mask"])

    w = {}
    sw = np.asarray(inputs["stem_w"], np.float32)
    w["stemW"] = np.ascontiguousarray(np.concatenate([sw, sw], axis=0)).astype(BF)  # (2F, DH) lhsT x2
    stem_b = np.asarray(inputs["stem_b"], np.float32)
    w["stemB2"] = np.ascontiguousarray(np.concatenate([stem_b, stem_b]).reshape(128, 1))
    cw = np.asarray(inputs["conv_w"], np.float32)  # (out, in, 3)
    cwt = cw.transpose(2, 1, 0)
    w["convW"] = np.ascontiguousarray(np.concatenate([cwt, cwt], axis=1)).astype(BF)  # (3, 2*in, out)
    conv_b = np.asarray(inputs["conv_b"], np.float32)
    w["convB2"] = np.ascontiguousarray(np.concatenate([conv_b, conv_b]).reshape(128, 1))
    wih = np.asarray(inputs["gru_w_ih"], np.float32)
    whh = np.asarray(inputs["gru_w_hh"], np.float32)
    bih = np.asarray(inputs["gru_b_ih"], np.float32)
    bhh = np.asarray(inputs["gru_b_hh"], np.float32)
    wihT = np.zeros((3, DH + 1, DG), np.float32)
    for g in range(3):
        wihT[g, :DH] = wih[g * DG:(g + 1) * DG].T
        bias = bih[g * DG:(g + 1) * DG].copy()
        if g < 2:
            bias += bhh[g * DG:(g + 1) * DG]
        wihT[g, DH] = bias
    w["wihT"] = wihT.astype(F8NP)
    w["whhT"] = np.ascontiguousarray(
        np.stack([whh[g * DG:(g + 1) * DG].T for g in range(3)])).astype(BF)
    w["bhhn"] = bhh[2 * DG:].reshape(DG, 1).astype(np.float32)
    w["ident"] = np.eye(128, dtype=np.float32).astype(BF)
    w["h1w"] = np.asarray(inputs["h1_w"], np.float32).astype(BF)
    w["h1b"] = np.asarray(inputs["h1_b"], np.float32).reshape(128, 1)
    w["h2w"] = np.asarray(inputs["h2_w"], np.float32).astype(BF)
    w["h2b"] = np.asarray(inputs["h2_b"], np.float32).reshape(128, 1)
    w["h3w"] = np.asarray(inputs["h3_w"], np.float32).astype(BF)  # (128, 64)
    h3b = np.asarray(inputs["h3_b"], np.float32)

    per_core = []
    for c in range(NCORE):
        rows = slice(c * B_C, (c + 1) * B_C)
        xc = x[rows]                       # (8, T, F)
        fmc = fm[rows]
        xm = np.where(fmc, 0.0, xc)
        d = dict(w)
        d["xmT"] = np.ascontiguousarray(xm.transpose(0, 2, 1)).astype(BF)
        mf = fmc.astype(np.float32)
        mxb = mf * (xc - h3b)              # (8, T, F)
        d["mmxP"] = _pack_mmx(mf, mxb)
        per_core.append(d)

    scale = np.std(x.astype(np.float64), axis=(0, 1), ddof=1) + 1e-8
    host_ctx = dict(inv_scale2=1.0 / (scale * scale),
                    msum=float(fm.sum()))
    return per_core, host_ctx


def host_finalize(core_outs, host_ctx):
    tot = np.sum([np.asarray(o, np.float64).sum(axis=1) for o in core_outs], axis=0)
    sf = tot[0:64] + tot[64:128]
    num = float(np.sum(sf * host_ctx["inv_scale2"]))
    den = max(host_ctx["msum"], 1.0)
    return np.float32(num / den)


def build_program():
    nc = bacc.Bacc("TRN2", target_bir_lowering=False, debug=False,
                   num_devices=NCORE)

    # ---- DRAM tensors ----
    xmT = nc.dram_tensor("xmT", [B_C, F, T], BF16, kind="ExternalInput").ap()
    stemW = nc.dram_tensor("stemW", [2 * F, DH], BF16, kind="ExternalInput").ap()
    stemB2 = nc.dram_tensor("stemB2", [128, 1], F32, kind="ExternalInput").ap()
    convW = nc.dram_tensor("convW", [3, 2 * DH, DH], BF16, kind="ExternalInput").ap()
    convB2 = nc.dram_tensor("convB2", [128, 1], F32, kind="ExternalInput").ap()
    wihT = nc.dram_tensor("wihT", [3, DH + 1, DG], F8, kind="ExternalInput").ap()
    whhT = nc.dram_tensor("whhT", [3, DG, DG], BF16, kind="ExternalInput").ap()
    bhhn = nc.dram_tensor("bhhn", [DG, 1], F32, kind="ExternalInput").ap()
    ident = nc.dram_tensor("ident", [128, 128], BF16, kind="ExternalInput").ap()
    h1w = nc.dram_tensor("h1w", [DG, 128], BF16, kind="ExternalInput").ap()
    h1b = nc.dram_tensor("h1b", [128, 1], F32, kind="ExternalInput").ap()
    h2w = nc.dram_tensor("h2w", [128, 128], BF16, kind="ExternalInput").ap()
    h2b = nc.dram_tensor("h2b", [128, 1], F32, kind="ExternalInput").ap()
    h3w = nc.dram_tensor("h3w", [128, F], BF16, kind="ExternalInput").ap()
    mmxP = nc.dram_tensor("mmxP", [2, 128, TOTBC], BF16, kind="ExternalInput").ap()
    convh = nc.dram_tensor("convh", [B_C, DH + 1, T], F8).ap()
    out = nc.dram_tensor("out", [128, NTILES], F32, kind="ExternalOutput").ap()

    with tile.TileContext(nc) as tc, ExitStack() as ctx:
        wpool = ctx.enter_context(tc.tile_pool(name="weights", bufs=1))
        spool = ctx.enter_context(tc.tile_pool(name="stats", bufs=1))

        def wtile(shape, src, tag, dt=BF16):
            t = wpool.tile(shape, dt, tag=tag)
            nc.gpsimd.dma_start(t[:], src)
            return t

        stemW_s = wtile([2 * F, DH], stemW[:], "w_stemW")
        stemB2_s = wtile([128, 1], stemB2[:], "w_stemB2", F32)
        convW_s = [wtile([2 * DH, DH], convW[dt], f"w_convW{dt}") for dt in range(3)]
        convB2_s = wtile([128, 1], convB2[:], "w_convB2", F32)
        wih_s = [wtile([DH + 1, DG], wihT[g], f"w_wih{g}", F8) for g in range(3)]
        whh_s = [wtile([DG, DG], whhT[g], f"w_whh{g}") for g in range(3)]
        bhhn_s = wtile([DG, 1], bhhn[:], "w_bhhn", F32)
        ident_s = wtile([128, 128], ident[:], "w_ident")
        h1w_s = wtile([DG, 128], h1w[:], "w_h1w")
        h1b_s = wtile([128, 1], h1b[:], "w_h1b", F32)
        h2w_s = wtile([128, 128], h2w[:], "w_h2w")
        h2b_s = wtile([128, 1], h2b[:], "w_h2b", F32)
        h3w_s = wtile([128, F], h3w[:], "w_h3w")

        # persistent slabs
        hout = wpool.tile([128, G, L, NB], BF16, tag="hout")
        zlane = wpool.tile([128, NB], BF16, tag="zlane")
        nc.vector.memset(zlane[:], 0.0)
        ones_sb = wpool.tile([128, T // 128], F8, tag="ones_sb")
        nc.vector.memset(ones_sb[:], 1.0)

        st_sf = spool.tile([128, NTILES], F32)
        nc.vector.memset(st_sf[:], 0.0)

        # ================= Phase A: encoder =================
        with tc.tile_pool(name="enc_io", bufs=3) as io, \
             tc.tile_pool(name="enc_row", bufs=1) as rowp, \
             tc.tile_pool(name="enc_ps", bufs=2, space="PSUM") as eps:
            for b in range(B_C):
                # ones row for the gx bias trick
                oview = convh[b, DH, 0:1].copy()
                oview.ap = _vecpairs([[T // 128, 128], [1, T // 128]])
                nc.gpsimd.dma_start(oview, ones_sb[:])
            CHB = 4 * TILE_T  # conv write burst (4 tiles)
            hrows, xslabs, chslabs = [], [], []
            for p in range(B_C // 2):
                hr = rowp.tile([128, T + 2], BF16, tag=f"hrow{p}", name=f"hrow{p}")
                nc.vector.memset(hr[:, 0:1], 0.0)
                nc.vector.memset(hr[:, T + 1:T + 2], 0.0)
                hrows.append(hr)
                xs = rowp.tile([128, T], BF16, tag=f"xslab{p}", name=f"xslab{p}")
                nc.sync.dma_start(xs[:], xmT[2 * p:2 * p + 2, :, :].rearrange("b c t -> (b c) t"))
                xslabs.append(xs)

            def conv_tile(p, it, chs):
                ps = eps.tile([128, TILE_T], F32, tag="conv_ps")
                for dt in range(3):
                    csl = slice(it * TILE_T + dt, it * TILE_T + dt + TILE_T)
                    nc.tensor.matmul(ps[0:64], convW_s[dt][0:64], hrows[p][0:64, csl],
                                     start=(dt == 0), stop=(dt == 2))
                    nc.tensor.matmul(ps[64:128], convW_s[dt][64:128], hrows[p][64:128, csl],
                                     start=(dt == 0), stop=(dt == 2))
                nc.scalar.activation(chs[:, (it % 4) * TILE_T:(it % 4 + 1) * TILE_T],
                                     ps[:], AF.Gelu, bias=convB2_s[:])
                if it % 4 == 3:
                    t0 = (it - 3) * TILE_T
                    nc.sync.dma_start(convh[2 * p, 0:DH, t0:t0 + CHB], chs[0:64])
                    nc.sync.dma_start(convh[2 * p + 1, 0:DH, t0:t0 + CHB], chs[64:128])

            chcur = [None] * (B_C // 2)
            for it in range(NT + 1):
                if it < NT:
                    tsl = ts(it, TILE_T)
                    for p in range(B_C // 2):
                        ps = eps.tile([128, TILE_T], F32, tag="stem_ps")
                        nc.tensor.matmul(ps[0:64], stemW_s[0:64], xslabs[p][0:64, tsl],
                                         start=True, stop=True)
                        nc.tensor.matmul(ps[64:128], stemW_s[64:128], xslabs[p][64:128, tsl],
                                         start=True, stop=True)
                        nc.scalar.activation(hrows[p][:, 1 + it * TILE_T:1 + (it + 1) * TILE_T],
                                             ps[:], AF.Gelu, bias=stemB2_s[:])
                if it >= 1:
                    for p in range(B_C // 2):
                        if (it - 1) % 4 == 0:
                            chcur[p] = rowp.tile([128, CHB], F8, tag=f"chslab{p}",
                                                 name=f"chs{p}_{it}", bufs=2)
                        conv_tile(p, it - 1, chcur[p])

        # ================= Phase B: chunk-parallel scan =================
        CW = L  # cin slab covers the whole scan
        with tc.tile_pool(name="scan_cin", bufs=1) as sin, \
             tc.tile_pool(name="scan_sm", bufs=3) as ssm, \
             tc.tile_pool(name="scan_ps", bufs=1, space="PSUM") as sps:
            # PSUM layout per group: [128, parity, slot, NB] f32 (2KB = 1 bank)
            P_rz = [sps.tile([128, 2, 2, NB], F32, tag=f"prz{g}", name=f"prz{g}") for g in range(G)]
            P_ngx = [sps.tile([128, 2, 2, NB], F32, tag=f"pngx{g}", name=f"pngx{g}") for g in range(G)]
            for g in range(G):
                nc.vector.memset(P_ngx[g][:, 0, 0, :], 0.0)  # P_n zero for t=0

            cins = [None] * G

            def load_cin(g):
                cin = wpool.tile([DH + 1, B_C, JPG, CW], F8, tag=f"cin{g}", name=f"cin{g}")
                for b in range(B_C):
                    src = convh[b, 0:DH + 1, 0:1].copy()
                    src.ap = _vecpairs([[T, DH + 1], [K, JPG], [1, L]])
                    src.offset = b * (DH + 1) * T + g * JPG * K
                    nc.sync.dma_start(cin[:, b, :, :], src)
                return cin

            def prefill(g, t):
                tp = t % 2
                if t == 0:
                    cins[g] = load_cin(g)
                rhs = cins[g][:, :, :, t]
                nc.tensor.matmul(P_rz[g][:, tp, 0, :], wih_s[0][:], rhs,
                                 start=True, stop=True, skip_group_check=True)
                nc.tensor.matmul(P_rz[g][:, tp, 1, :], wih_s[1][:], rhs,
                                 start=True, stop=True, skip_group_check=True)
                nc.tensor.matmul(P_ngx[g][:, tp, 1, :], wih_s[2][:], rhs,
                                 start=True, stop=True, skip_group_check=True)

            for g in range(G):
                prefill(g, 0)
            prev_n = [None] * G
            prev_zd = [None] * G
            for t in range(L):
                cp = t % 2
                if t + 1 < L:
                    for g in range(G):
                        prefill(g, t + 1)
                if t > 0:
                    for g in range(G):
                        # recurrent state fed as two operands: h = n + zd
                        nc.tensor.matmul(P_rz[g][:, cp, 0, :], whh_s[0][:], prev_n[g][:],
                                         start=False, stop=True, skip_group_check=True)
                        nc.tensor.matmul(P_rz[g][:, cp, 1, :], whh_s[1][:], prev_n[g][:],
                                         start=False, stop=True, skip_group_check=True)
                        nc.tensor.matmul(P_ngx[g][:, cp, 0, :], whh_s[2][:], prev_n[g][:],
                                         start=True, stop=False, skip_group_check=True)
                        nc.tensor.matmul(P_rz[g][:, cp, 0, :], whh_s[0][:], prev_zd[g][:],
                                         start=False, stop=True, skip_group_check=True)
                        nc.tensor.matmul(P_rz[g][:, cp, 1, :], whh_s[1][:], prev_zd[g][:],
                                         start=False, stop=True, skip_group_check=True)
                        nc.tensor.matmul(P_ngx[g][:, cp, 0, :], whh_s[2][:], prev_zd[g][:],
                                         start=False, stop=True, skip_group_check=True)
                for g in range(G):
                    h_prev = zlane[:] if t == 0 else hout[:, g, t - 1, :]
                    rz_sb = ssm.tile([128, 2, NB], BF16, tag=f"rz{g}")
                    nc.scalar.activation(rz_sb[:], P_rz[g][:, cp, :, :], AF.Sigmoid)
                    # tmp = (P_n + bhhn) * r
                    tmp = ssm.tile([128, NB], BF16, tag=f"tmp{g}")
                    nc.vector.scalar_tensor_tensor(tmp[:], P_ngx[g][:, cp, 0, :],
                                                   bhhn_s[:], rz_sb[:, 0, :],
                                                   ALU.add, ALU.mult)
                    if IMM_ON_PE:
                        nc.tensor.matmul(P_ngx[g][:, cp, 1, :], ident_s[:], tmp[:],
                                         start=False, stop=True, skip_group_check=True)
                    else:
                        nc.vector.tensor_add(P_ngx[g][:, cp, 1, :],
                                             P_ngx[g][:, cp, 1, :], tmp[:])
                    n_sb = ssm.tile([128, NB], BF16, tag=f"n{g}")
                    nc.scalar.activation(n_sb[:], P_ngx[g][:, cp, 1, :], AF.Tanh)
                    d_sb = ssm.tile([128, NB], BF16, tag=f"d{g}")
                    nc.vector.tensor_sub(d_sb[:], h_prev, n_sb[:])
                    zd_sb = ssm.tile([128, NB], BF16, tag=f"zd{g}")
                    nc.vector.tensor_mul(zd_sb[:], rz_sb[:, 1, :], d_sb[:])
                    nc.gpsimd.tensor_tensor(hout[:, g, t, :], n_sb[:], zd_sb[:], ALU.add)
                    prev_n[g], prev_zd[g] = n_sb, zd_sb

        # ================= Phase C: head + loss =================
        with tc.tile_pool(name="head_io", bufs=3) as hio, \
             tc.tile_pool(name="head_tmp", bufs=3) as htmp, \
             tc.tile_pool(name="head_ps", bufs=2, space="PSUM") as hps:
            def mlp(zview, ncols, p3dst):
                '''h1->gelu->h2->gelu->h3 into the given PSUM [64-row] dest.'''
                p1 = hps.tile([128, WMAX], F32, tag="p1")
                nc.tensor.matmul(p1[:, 0:ncols], h1w_s[:], zview, start=True, stop=True)
                r1 = htmp.tile([128, WMAX], BF16, tag="r1")
                nc.scalar.activation(r1[:, 0:ncols], p1[:, 0:ncols], AF.Gelu, bias=h1b_s[:])
                p2 = hps.tile([128, WMAX], F32, tag="p2")
                nc.tensor.matmul(p2[:, 0:ncols], h2w_s[:], r1[:, 0:ncols], start=True, stop=True)
                r2 = htmp.tile([128, WMAX], BF16, tag="r2")
                nc.scalar.activation(r2[:, 0:ncols], p2[:, 0:ncols], AF.Gelu, bias=h2b_s[:])
                nc.tensor.matmul(p3dst, h3w_s[:], r2[:, 0:ncols], start=True, stop=True)

            def loss_block(p3, width, mslab, moff, tidx):
                t1 = htmp.tile([128, WMAX], F32, tag="t1")
                nc.vector.tensor_mul(t1[:, 0:width], p3, mslab[:, 0, :, moff:moff + width].squeeze())
                dd = htmp.tile([128, WMAX], BF16, tag="dd")
                nc.vector.tensor_sub(dd[:, 0:width], t1[:, 0:width],
                                     mslab[:, 1, :, moff:moff + width].squeeze())
                sq = htmp.tile([128, WMAX], BF16, tag="junk")
                nc.vector.tensor_mul(sq[:, 0:width], dd[:, 0:width], dd[:, 0:width])
                nc.vector.tensor_reduce(st_sf[:, tidx:tidx + 1], sq[:, 0:width],
                                        mybir.AxisListType.X, ALU.add)

            def zview_of(g, nb0, nl):
                return hout[:, g, W:L, nb0:nb0 + nl].rearrange("p t n -> p n t")

            tidx = 0
            for g in range(G):
                gw = GBC + (EXTRA_W if g == G - 1 else 0)
                mslab = hio.tile([128, 2, 1, gw], BF16, tag="mslab", bufs=2)
                nc.sync.dma_start(mslab[:], mmxP[:, :, g * GBC:g * GBC + gw].rearrange(
                    "a p c -> p a () c"))
                moff = 0
                for i in range(10):  # stacked pairs
                    p3 = hps.tile([128, WMAX], F32, tag="p3")
                    mlp(zview_of(g, 2 * i * HL, HL), WMAX, p3[0:64, :])
                    mlp(zview_of(g, (2 * i + 1) * HL, HL), WMAX, p3[64:128, :])
                    loss_block(p3[:], WMAX, mslab, moff, tidx)
                    moff += WMAX
                    tidx += 1
                # singles: block 20 (full) and block 21 (short)
                for nb0, wsub in ((20 * HL, WMAX), (21 * HL, (NB - 21 * HL) * K)):
                    p3 = hps.tile([128, WMAX], F32, tag="p3")
                    nc.vector.memset(p3[64:128, 0:wsub], 0.0)
                    mlp(zview_of(g, nb0, (wsub // K)), wsub, p3[0:64, 0:wsub])
                    loss_block(p3[:, 0:wsub], wsub, mslab, moff, tidx)
                    moff += wsub
                    tidx += 1
            # j0-extra single
            p3 = hps.tile([128, WMAX], F32, tag="p3")
            nc.vector.memset(p3[64:128, 0:EXTRA_W], 0.0)
            zv = hout[:, 0, 0, 0:1].copy()
            zv.ap = _vecpairs([list(hout.ap[0]), [JPG, B_C], [NB, W]])
            zv.offset = hout.offset
            mlp(zv, EXTRA_W, p3[0:64, 0:EXTRA_W])
            loss_block(p3[:, 0:EXTRA_W], EXTRA_W, mslab, moff, tidx)
            tidx += 1
            nc.sync.dma_start(out[:], st_sf[:])

    nc.compile()
    return nc


_CACHE = {}


def kernel(**inputs):
    """Full-input entry point: shards over 8 NeuronCores, runs the Bass
    program, returns the scalar loss (np.float32)."""
    from concourse.bass_utils import run_bass_kernel_spmd

    if "nc" not in _CACHE:
        _CACHE["nc"] = build_program()
    nc = _CACHE["nc"]
    per_core, host_ctx = prep_inputs(inputs)
    res = run_bass_kernel_spmd(nc, per_core, list(range(NCORE))).results
    return host_finalize([r["out"] for r in res], host_ctx)
